# revision 10
# baseline (speedup 1.0000x reference)
"""AlexNet-style CNN forward pass on 8 Trainium2 NeuronCores.

Strategy:
  - Convs data-parallel: batch 256 -> 32 per core, channels on partitions,
    conv = sum of shifted matmuls over kernel offsets (weights replicated).
  - conv1 (cin=3) uses host-packed im2col rows (3 dy-offsets x 11 dx x 3 ch
    + ones row for fused bias -> K=100) so the PE array is well utilized.
  - conv2 uses an x-shifted duplicate of its input (K=128 = 2 dx-offsets
    x 64 ch) to fill the contraction dim.
  - FC layers model-parallel: each core owns 512 rows of fc1/fc2 and 512
    K-columns of fc3; activations are AllGathered between layers, fc3
    partials AllReduced.  This cuts per-core FC weight DMA 8x.
  - Matmuls/activations run in bf16 (halves DMA + PE power so the clock
    stays unthrottled); PSUM accumulation + biases + fc3 AllReduce in fp32.
"""

import numpy as np
import ml_dtypes

BF = ml_dtypes.bfloat16

import concourse.bass as bass
import concourse.mybir as mybir
import concourse.tile as tile
from concourse import bacc
from concourse.bass_utils import run_bass_kernel_spmd

N_CORES = 8
B = 256
BC = B // N_CORES  # 32 images per core

F32 = mybir.dt.float32
BF16 = mybir.dt.bfloat16
RELU = mybir.ActivationFunctionType.Relu
IDENT = mybir.ActivationFunctionType.Identity


def _emit(nc, tc, t, yout):
    """Emit the whole network. t: dict name -> DRAM AP."""
    sync = nc.sync
    act = nc.scalar
    dve = nc.vector
    pool_e = nc.gpsimd

    psum = tc.alloc_tile_pool(name="psum", bufs=6, space="PSUM")
    scr = tc.alloc_tile_pool(name="scr", bufs=1, side="left")
    dram = tc.alloc_tile_pool(name="dram", bufs=1, space="DRAM")

    # ---------------- phase pools (queue alloc mode handles overlap) ----
    p_w12 = tc.alloc_tile_pool(name="p_w12", bufs=1, side="left")
    p_x2s = tc.alloc_tile_pool(name="p_x2s", bufs=1, side="left")
    p_x13 = tc.alloc_tile_pool(name="p_x13", bufs=3, side="left")

    # conv1+conv2 weights (host arrays already in SBUF layout).
    # Weights ride the Activation HWDGE queue so they never
    # head-of-line-block the x13 input stream on the SP queue.
    # conv1/conv2-mc1 weights have out-channels duplicated to M=128: the
    # "wasted" PE columns produce a second copy of the output, which the
    # maxpool writes at an x-offset of -1 -- giving conv2/conv3 their
    # x+1-shifted K-packing copies without any partition-shift DMA.
    lw1 = p_w12.tile([100, 4 * 128], BF16)
    act.dma_start(lw1[:], t["lw1"][:])
    lw2 = p_w12.tile([128, 7 * 4 * 256], BF16)
    act.dma_start(lw2[:], t["lw2"][:])
    lb2 = p_w12.tile([128, 2], F32)
    act.dma_start(lb2[:], t["lb2"][:])

    # warmup collectives: the first op of each replica-group shape pays a
    # ~40-100us cold-start; absorb them here, overlapped with conv1
    QUADS = [[0, 1, 2, 3], [4, 5, 6, 7]]
    PAIRS = [[0, 4], [1, 5], [2, 6], [3, 7]]
    wg_i = dram.tile([128, 8], BF16)
    wg_o = dram.tile([4, 128, 8], BF16)
    pool_e.collective_compute(
        "AllGather", mybir.AluOpType.bypass,
        replica_groups=QUADS, ins=[wg_i.opt()], outs=[wg_o.opt()])
    wr_i = dram.tile([128, 8], F32)
    wr_o = dram.tile([128, 8], F32)
    pool_e.collective_compute(
        "AllReduce", mybir.AluOpType.add,
        replica_groups=QUADS, ins=[wr_i.opt()], outs=[wr_o.opt()])
    wp_i = dram.tile([128, 8], F32)
    wp_o = dram.tile([2, 128, 8], F32)
    pool_e.collective_compute(
        "AllGather", mybir.AluOpType.bypass,
        replica_groups=PAIRS, ins=[wp_i.opt()], outs=[wp_o.opt()])

    # conv2 input: [128, BC, 22, 23]; rows 0:64 ch c at x, rows 64:128 ch c at x+1
    X2s = p_x2s.tile([128, BC * 22 * 23], BF16)
    pool_e.memset(X2s[:], 0.0)

    def x2v(p0, p1, b0, nb, y0, ny, x0, nx):
        return X2s[p0:p1].rearrange("p (b y x) -> p b y x", b=BC, y=22, x=23)[
            :, b0:b0 + nb, y0:y0 + ny, x0:x0 + nx]

    # ---------------- conv1 + pool1 ----------------
    _sid = nc.enter_named_scope("L1_conv1", False)[0]
    for bg in range(4):  # groups of 8 images
        xt = p_x13.tile([100, 8 * 41 * 32], BF16, tag="x13")
        sync.dma_start(xt[:], t["x13"][bg])
        xtv = xt.rearrange("k (b y x) -> k b y x", b=8, y=41, x=32)
        for bl in range(8):
            b = bg * 8 + bl
            for h in range(2):  # vertical half of the 32x32 output
                ps = psum.tile([128, 512], F32, tag="ps")
                for pi, p in enumerate((0, 3, 6, 9)):
                    nc.tensor.matmul(
                        ps[:],
                        lw1[:, pi * 128:(pi + 1) * 128],
                        xtv[:, bl, h * 16 + p:h * 16 + p + 16, :],
                        start=(pi == 0), stop=(pi == 3),
                    )
                # evict+relu (bias came in via the ones-row), then 2x2 maxpool
                s1 = scr.tile([128, 512], BF16, tag="ev", bufs=3)
                act.activation(s1[:, :], ps[:], RELU)
                s1v = s1.rearrange("m (y x) -> m y x", y=16, x=32)
                m1 = scr.tile([128, 128], BF16, tag="m1", bufs=2)
                m2 = scr.tile([128, 128], BF16, tag="m2", bufs=2)
                dve.tensor_max(m1[:], s1v[:, 0::2, 0::2], s1v[:, 0::2, 1::2])
                dve.tensor_max(m2[:], s1v[:, 1::2, 0::2], s1v[:, 1::2, 1::2])
                y0 = h * 8 + 3
                m1v = m1.rearrange("m (y x) -> m y x", y=8, x=16)
                m2v = m2.rearrange("m (y x) -> m y x", y=8, x=16)
                dve.tensor_max(
                    x2v(0, 64, b, 1, y0, 8, 3, 16)[:, 0],
                    m1v[0:64], m2v[0:64])
                dve.tensor_max(
                    x2v(64, 128, b, 1, y0, 8, 2, 16)[:, 0],
                    m1v[64:128], m2v[64:128])
    p_x13.release()
    nc.leave_named_scope("L1_conv1", _sid, False)

    # conv3 weights (prefetch during conv2) + conv3 input buffers
    p_w3 = tc.alloc_tile_pool(name="p_w3", bufs=1, side="right")
    p_x3 = tc.alloc_tile_pool(name="p_x3", bufs=1, side="right")
    lw3 = p_w3.tile([128, 15360], BF16)
    act.dma_start(lw3[:], t["lw3"][:])
    lb3 = p_w3.tile([128, 3], F32)
    act.dma_start(lb3[:], t["lb3"][:])
    X3a = p_x3.tile([128, BC * 12 * 12], BF16)
    # X3b rows 64:128 hold the x+1-shifted copy of rows 0:64 (written
    # directly by conv2's duplicated-M maxpool), so kc1 runs as K=128
    # matmuls covering kernel-offset pairs (dy,2j)+(dy,2j+1)
    X3b = p_x3.tile([128, BC * 12 * 12], BF16)
    pool_e.memset(X3a[:], 0.0)
    pool_e.memset(X3b[:], 0.0)

    def x3v(xab, p0, p1, b0, nb, y0, ny, x0, nx):
        return xab[p0:p1].rearrange("p (b y x) -> p b y x", b=BC, y=12, x=12)[
            :, b0:b0 + nb, y0:y0 + ny, x0:x0 + nx]

    # ---------------- conv2 + pool2 ----------------
    _sid = nc.enter_named_scope("L2_conv2", False)[0]
    lw2v = lw2.rearrange("k (a j m) -> k a j m", a=7, j=4, m=256)
    for nt in range(16):  # pairs of images
        for mc in range(2):
            ps = psum.tile([128, 512], F32, tag="ps")
            first = True
            for dy in range(7):
                for j in range(4):
                    K = 128 if j < 3 else 64
                    xoff = 2 * j if j < 3 else 6
                    nc.tensor.matmul(
                        ps[:],
                        lw2v[0:K, dy, j, mc * 128:mc * 128 + 128],
                        x2v(0, K, nt * 2, 2, dy, 16, xoff, 16),
                        start=first, stop=(dy == 6 and j == 3),
                    )
                    first = False
            s2 = scr.tile([128, 512], BF16, tag="ev", bufs=3)
            act.activation(s2[:], ps[:], RELU, bias=lb2[:, mc:mc + 1])
            s2v = s2.rearrange("m (b y x) -> m b y x", b=2, y=16, x=16)
            m1 = scr.tile([128, 128], BF16, tag="m1", bufs=2)
            m2 = scr.tile([128, 128], BF16, tag="m2", bufs=2)
            dve.tensor_max(m1[:], s2v[:, :, 0::2, 0::2], s2v[:, :, 0::2, 1::2])
            dve.tensor_max(m2[:], s2v[:, :, 1::2, 0::2], s2v[:, :, 1::2, 1::2])
            m1v = m1.rearrange("m (b y x) -> m b y x", b=2, y=8, x=8)
            m2v = m2.rearrange("m (b y x) -> m b y x", b=2, y=8, x=8)
            if mc == 0:
                dve.tensor_max(x3v(X3a, 0, 128, nt * 2, 2, 2, 8, 2, 8), m1v[:], m2v[:])
            else:
                # ch 128:192 at (2,2) on rows 0:64; the duplicated copy goes
                # to rows 64:128 at x-offset 1 => x+1-shifted for kc1 pairing
                dve.tensor_max(x3v(X3b, 0, 64, nt * 2, 2, 2, 8, 2, 8),
                               m1v[0:64], m2v[0:64])
                dve.tensor_max(x3v(X3b, 64, 128, nt * 2, 2, 2, 8, 1, 8),
                               m1v[64:128], m2v[64:128])
    nc.leave_named_scope("L2_conv2", _sid, False)
    p_x2s.release()
    p_w12.release()

    # conv4/5 weights (prefetch during conv3) + conv4 input buffers
    p_w45 = tc.alloc_tile_pool(name="p_w45", bufs=1, side="left")
    p_x4 = tc.alloc_tile_pool(name="p_x4", bufs=1, side="left")
    lw4 = p_w45.tile([128, 27 * 256], BF16)
    act.dma_start(lw4[:], t["lw4"][:])
    lb4 = p_w45.tile([128, 2], F32)
    act.dma_start(lb4[:], t["lb4"][:])
    lw5 = p_w45.tile([128, 18 * 256], BF16)
    act.dma_start(lw5[:], t["lw5"][:])
    lb5 = p_w45.tile([128, 2], F32)
    act.dma_start(lb5[:], t["lb5"][:])
    X4 = []
    for i in range(3):
        X4.append(p_x4.tile([128, BC * 10 * 10], BF16, name=f"X4_{i}"))
        pool_e.memset(X4[i][:], 0.0)

    def xv10(xab, p0, p1, b0, nb, y0, ny, x0, nx):
        return xab[p0:p1].rearrange("p (b y x) -> p b y x", b=BC, y=10, x=10)[
            :, b0:b0 + nb, y0:y0 + ny, x0:x0 + nx]

    _sid = nc.enter_named_scope("L3_conv3", False)[0]
    # ---------------- conv3 ----------------
    for nt in range(4):  # 8 images
        for mc in range(3):
            ps = psum.tile([128, 512], F32, tag="ps")
            first = True
            for dy in range(5):
                for dx in range(5):
                    blk = dy * 5 + dx
                    nc.tensor.matmul(
                        ps[:],
                        lw3[0:128, blk * 384 + mc * 128:blk * 384 + mc * 128 + 128],
                        x3v(X3a, 0, 128, nt * 8, 8, dy, 8, dx, 8),
                        start=first, stop=False,
                    )
                    first = False
            for dy in range(5):
                for j in range(2):
                    base = 9600 + (dy * 2 + j) * 384 + mc * 128
                    nc.tensor.matmul(
                        ps[:],
                        lw3[0:128, base:base + 128],
                        x3v(X3b, 0, 128, nt * 8, 8, dy, 8, 2 * j, 8),
                        start=False, stop=False,
                    )
                base = 13440 + dy * 384 + mc * 128
                nc.tensor.matmul(
                    ps[:],
                    lw3[0:64, base:base + 128],
                    x3v(X3b, 0, 64, nt * 8, 8, dy, 8, 4, 8),
                    start=False, stop=(dy == 4),
                )
            act.activation(
                xv10(X4[mc], 0, 128, nt * 8, 8, 1, 8, 1, 8),
                ps.rearrange("m (b y x) -> m b y x", b=8, y=8, x=8),
                RELU, bias=lb3[:, mc:mc + 1])
    nc.leave_named_scope("L3_conv3", _sid, False)
    p_x3.release()
    p_w3.release()

    # fc1 weights (prefetch during conv4) + conv5 input buffers
    p_fw1 = tc.alloc_tile_pool(name="p_fw1", bufs=1, side="right")
    p_fw2 = tc.alloc_tile_pool(name="p_fw2", bufs=1, side="right")
    p_x5 = tc.alloc_tile_pool(name="p_x5", bufs=1, side="right")
    fw1 = p_fw1.tile([128, 32 * 1024], BF16)
    act.dma_start(fw1[:], t["fw1s"][:])
    fb1 = p_fw1.tile([128, 8], F32)
    act.dma_start(fb1[:], t["fb1s"][:])
    fw2 = p_fw2.tile([128, 32 * 1024], BF16)
    act.dma_start(fw2[:], t["fw2s"][:])
    fb2 = p_fw2.tile([128, 8], F32)
    act.dma_start(fb2[:], t["fb2s"][:])
    fw3 = p_fw2.tile([128, 8 * 100], BF16)
    act.dma_start(fw3[:], t["fw3s"][:])
    fb3 = p_fw2.tile([100, 1], F32)
    act.dma_start(fb3[:], t["fb3s"][:])
    X5 = []
    for i in range(2):
        X5.append(p_x5.tile([128, BC * 10 * 10], BF16, name=f"X5_{i}"))
        pool_e.memset(X5[i][:], 0.0)

    _sid = nc.enter_named_scope("L4_conv4", False)[0]
    # ---------------- conv4 ----------------
    lw4v = lw4.rearrange("k (o m) -> k o m", o=27)
    for nt in range(4):
        for mc in range(2):
            ps = psum.tile([128, 512], F32, tag="ps")
            first = True
            for dy in range(3):
                for dx in range(3):
                    for kc in range(3):
                        o = (dy * 3 + dx) * 3 + kc
                        nc.tensor.matmul(
                            ps[:],
                            lw4v[:, o, mc * 128:mc * 128 + 128],
                            xv10(X4[kc], 0, 128, nt * 8, 8, dy, 8, dx, 8),
                            start=first, stop=(o == 26),
                        )
                        first = False
            act.activation(
                xv10(X5[mc], 0, 128, nt * 8, 8, 1, 8, 1, 8),
                ps.rearrange("m (b y x) -> m b y x", b=8, y=8, x=8),
                RELU, bias=lb4[:, mc:mc + 1])
    nc.leave_named_scope("L4_conv4", _sid, False)
    p_x4.release()

    # pool5 output, separate tiles per image-half so the half-0 gather
    # trigger doesn't wait on half-1 pool writes
    p_p5 = tc.alloc_tile_pool(name="p_p5", bufs=1, side="left")
    P5 = [[p_p5.tile([128, (BC // 2) * 16], BF16, name=f"P5_{h}_{i}")
           for i in range(2)] for h in range(2)]

    # DRAM staging for the pipelined gather/FC tail.  FC layers are
    # sharded over 4-core quads (cores 0-3 already hold images 0:128, so
    # each quad only gathers ITS half of the batch -- half the collective
    # bytes, and the two quads' collectives run concurrently).  Within a
    # quad each core owns 1024 fc1/fc2 rows and 1024 fc3 K-columns.  A
    # final pair AllGather swaps logits across quads.  Everything is also
    # split into 2 image-halves so collectives overlap fc compute.
    HB = BC // 2          # 16 local images per half
    NHQ = 4 * HB          # 64 quad images per half
    cin5 = [dram.tile([2, 128, HB * 16], BF16, name=f"cin5_{h}") for h in range(2)]
    g1 = [dram.tile([4, 2, 128, HB * 16], BF16, name=f"g1_{h}")
          for h in range(2)]

    def stage_gather1(h):
        for mc in range(2):
            sync.dma_start(cin5[h][mc], P5[h][mc][:])
        pool_e.collective_compute(
            "AllGather", mybir.AluOpType.bypass,
            replica_groups=QUADS, ins=[cin5[h].opt()], outs=[g1[h].opt()])

    _sid = nc.enter_named_scope("L5_conv5", False)[0]
    # ---------------- conv5 + pool5 ----------------
    lw5v = lw5.rearrange("k (o m) -> k o m", o=18)
    for nt in range(4):
        for mc in range(2):
            ps = psum.tile([128, 512], F32, tag="ps")
            first = True
            for dy in range(3):
                for dx in range(3):
                    for kc in range(2):
                        o = (dy * 3 + dx) * 2 + kc
                        nc.tensor.matmul(
                            ps[:],
                            lw5v[:, o, mc * 128:mc * 128 + 128],
                            xv10(X5[kc], 0, 128, nt * 8, 8, dy, 8, dx, 8),
                            start=first, stop=(o == 17),
                        )
                        first = False
            s5 = scr.tile([128, 512], BF16, tag="ev", bufs=3)
            act.activation(s5[:], ps[:], RELU, bias=lb5[:, mc:mc + 1])
            s5v = s5.rearrange("m (b y x) -> m b y x", b=8, y=8, x=8)
            m1 = scr.tile([128, 128], BF16, tag="m1", bufs=2)
            m2 = scr.tile([128, 128], BF16, tag="m2", bufs=2)
            dve.tensor_max(m1[:], s5v[:, :, 0::2, 0::2], s5v[:, :, 0::2, 1::2])
            dve.tensor_max(m2[:], s5v[:, :, 1::2, 0::2], s5v[:, :, 1::2, 1::2])
            p5v = P5[nt // 2][mc].rearrange("p (b y x) -> p b y x", b=HB, y=4, x=4)
            bo = (nt % 2) * 8
            dve.tensor_max(
                p5v[:, bo:bo + 8, :, :],
                m1.rearrange("m (b y x) -> m b y x", b=8, y=4, x=4),
                m2.rearrange("m (b y x) -> m b y x", b=8, y=4, x=4))
        if nt == 1:
            stage_gather1(0)
    stage_gather1(1)
    nc.leave_named_scope("L5_conv5", _sid, False)
    p_x5.release()
    p_p5.release()
    p_w45.release()

    _sid = nc.enter_named_scope("G1_gather", False)[0]
    # un-interleave gathered pool5 into SBUF, then re-layout so each fc1
    # K-step reads a CONTIGUOUS rhs block (strided cols cross an SBUF
    # cacheline per column and throttle PE streaming ~4x)
    p_h1 = tc.alloc_tile_pool(name="p_h1", bufs=1, side="right")
    H1 = [[p_h1.tile([128, 4 * HB * 16], BF16, name=f"H1_{h}_{cc}")
           for cc in range(2)] for h in range(2)]
    H1y = [p_h1.tile([128, 32 * NHQ], BF16, name=f"H1y_{h}") for h in range(2)]
    for h in range(2):
        for cc in range(2):
            sync.dma_start(
                H1[h][cc].rearrange("c (r f) -> c r f", r=4),
                g1[h][:, cc].rearrange("r c f -> c r f"))
        for cc in range(2):
            dve.tensor_copy(
                H1y[h].rearrange("c (y s q) -> c y s q", y=16, s=2)[:, :, cc],
                H1[h][cc].rearrange("c (q y) -> c y q", y=16))
    nc.leave_named_scope("G1_gather", _sid, False)

    _sid = nc.enter_named_scope("F1_fc1", False)[0]
    # ------------- fc1 (quad-model-parallel over 1024 outputs) -------------
    p_f1 = tc.alloc_tile_pool(name="p_f1", bufs=1, side="left")
    F1 = [p_f1.tile([128, 8 * NHQ], BF16, name=f"F1_{h}") for h in range(2)]
    cin6 = [dram.tile([128, 8 * NHQ], BF16, name=f"cin6_{h}") for h in range(2)]
    g2 = [dram.tile([4, 128, 8 * NHQ], BF16, name=f"g2_{h}")
          for h in range(2)]
    fw1v = fw1.rearrange("k (y c m) -> k y c m", y=16, c=2, m=1024)
    for h in range(2):
        rhsv = H1y[h].rearrange("c (y s q) -> c y s q", y=16, s=2)
        for mc in range(8):
            ps = psum.tile([128, NHQ], F32, tag="ps")
            first = True
            for yx in range(16):
                for cc in range(2):
                    nc.tensor.matmul(
                        ps[:],
                        fw1v[:, yx, cc, mc * 128:mc * 128 + 128],
                        rhsv[:, yx, cc],
                        start=first, stop=(yx == 15 and cc == 1))
                    first = False
            act.activation(F1[h][:, mc * NHQ:(mc + 1) * NHQ], ps[:], RELU,
                           bias=fb1[:, mc:mc + 1])
        sync.dma_start(cin6[h][:], F1[h][:])
        pool_e.collective_compute(
            "AllGather", mybir.AluOpType.bypass,
            replica_groups=QUADS, ins=[cin6[h].opt()], outs=[g2[h].opt()])
    p_h1.release()
    nc.leave_named_scope("F1_fc1", _sid, False)

    _sid = nc.enter_named_scope("G2_gather", False)[0]
    # un-interleave gathered fc1: feature block kc = rq*8 + mc, so the
    # (rq, mc, b) column order is already kc-major
    p_h2 = tc.alloc_tile_pool(name="p_h2", bufs=1, side="right")
    H2 = [p_h2.tile([128, 32 * NHQ], BF16, name=f"H2_{h}") for h in range(2)]
    for h in range(2):
        sync.dma_start(
            H2[h].rearrange("c (r f) -> c r f", r=4),
            g2[h].rearrange("r c f -> c r f"))
    p_f1.release()
    nc.leave_named_scope("G2_gather", _sid, False)

    _sid = nc.enter_named_scope("F2_fc23", False)[0]
    # ---------------- fc2 + fc3 partial + quad AllReduce ----------------
    p_f2 = tc.alloc_tile_pool(name="p_f2", bufs=1, side="left")
    F2 = [p_f2.tile([128, 8 * NHQ], BF16, name=f"F2_{h}") for h in range(2)]
    cin7 = [dram.tile([100, NHQ], F32, name=f"cin7_{h}") for h in range(2)]
    g3 = [dram.tile([100, NHQ], F32, name=f"g3_{h}") for h in range(2)]
    gf = [dram.tile([2, 100, NHQ], F32, name=f"gf_{h}") for h in range(2)]
    youtv = yout.rearrange("m (q rr g) -> m q rr g", q=2, rr=4)
    fw2v = fw2.rearrange("k (a m) -> k a m", a=32)
    fw3v = fw3.rearrange("k (a m) -> k a m", a=8)
    for h in range(2):
        for mc in range(8):
            ps = psum.tile([128, NHQ], F32, tag="ps")
            for kc in range(32):
                nc.tensor.matmul(
                    ps[:], fw2v[:, kc, mc * 128:mc * 128 + 128],
                    H2[h][:, kc * NHQ:(kc + 1) * NHQ],
                    start=(kc == 0), stop=(kc == 31))
            act.activation(F2[h][:, mc * NHQ:(mc + 1) * NHQ], ps[:], RELU,
                           bias=fb2[:, mc:mc + 1])
        ps3 = psum.tile([128, NHQ], F32, tag="ps")
        for kc in range(8):
            nc.tensor.matmul(
                ps3[0:100, :], fw3v[:, kc, :], F2[h][:, kc * NHQ:(kc + 1) * NHQ],
                start=(kc == 0), stop=(kc == 7))
        s3 = scr.tile([128, 512], F32, tag="ev", bufs=3)
        act.activation(s3[0:100, 0:NHQ], ps3[0:100, :], IDENT, bias=fb3[:])
        sync.dma_start(cin7[h][:], s3[0:100, 0:NHQ])
        pool_e.collective_compute(
            "AllReduce", mybir.AluOpType.add,
            replica_groups=QUADS, ins=[cin7[h].opt()], outs=[g3[h].opt()])
        # swap quad logits across quads: rank order in each pair = quad order
        pool_e.collective_compute(
            "AllGather", mybir.AluOpType.bypass,
            replica_groups=PAIRS, ins=[g3[h].opt()], outs=[gf[h].opt()])
        # yout col = q*128 + rq*32 + h*16 + b  (split per quad: DMA APs
        # only support 3 dims after balancing)
        for q in range(2):
            sync.dma_start(
                youtv[:, q, :, h * HB:(h + 1) * HB],
                gf[h][q].rearrange("m (rr b) -> m rr b", rr=4))
    nc.leave_named_scope("F2_fc23", _sid, False)
    p_f2.release()
    p_h2.release()
    p_fw2.release()
    p_fw1.release()

    scr.release()
    dram.release()
    psum.release()


# ---------------------------------------------------------------------------
# host-side input prep (numpy; all weight arrays already in SBUF layout)
# ---------------------------------------------------------------------------

def _prep_shared(w1, b1, w2, b2, w3, b3, w4, b4, w5, b5):
    f = np.float32
    # conv1: rows r = dyo*33 + dx*3 + c, row 99 = bias(ones); 4 passes
    # dy=3p+dyo; out-channels duplicated to M=128 (cols 64:128 = cols 0:64)
    lw1 = np.zeros((100, 4 * 128), f)
    for p in range(4):
        for dyo in range(3):
            dy = 3 * p + dyo
            if dy > 10:
                continue
            for dx in range(11):
                for c in range(3):
                    lw1[dyo * 33 + dx * 3 + c, p * 128:p * 128 + 64] = w1[:, c, dy, dx]
    lw1[99, 0:64] = b1
    lw1[:, :] = lw1.reshape(100, 4, 2, 64)[:, :, 0:1].repeat(2, 2).reshape(100, -1)
    # conv2: [128, (dy,j,m)] m=256: j<3 -> rows s*64+c = ch c at dx=2j+s;
    # j=3 -> dx=6.  m 0:128 = out-ch 0:128; m 128:256 = out-ch 128:192 twice
    lw2 = np.zeros((128, 7 * 4 * 256), f)
    w2m = np.concatenate([w2[0:128], w2[128:192], w2[128:192]], axis=0)  # [256,...]
    for dy in range(7):
        for j in range(3):
            for s in range(2):
                lw2[s * 64:(s + 1) * 64, (dy * 4 + j) * 256:(dy * 4 + j + 1) * 256] = \
                    w2m[:, :, dy, 2 * j + s].T
        lw2[0:64, (dy * 4 + 3) * 256:(dy * 4 + 4) * 256] = w2m[:, :, dy, 6].T
    lb2 = np.zeros((128, 2), f)
    lb2[:, 0] = b2[0:128]
    lb2[0:64, 1] = b2[128:192]
    lb2[64:128, 1] = b2[128:192]
    # conv3: [128, 15360]: kc0 cols blk*384+m; kc1 pairs (dy,2j)+(dy,2j+1)
    # at 9600+(dy*2+j)*384 rows [0:64|64:128]; kc1 singles (dy,4) at
    # 13440+dy*384 rows 0:64
    lw3 = np.zeros((128, 15360), f)
    for dy in range(5):
        for dx in range(5):
            blk = dy * 5 + dx
            lw3[:, blk * 384:(blk + 1) * 384] = w3[:, 0:128, dy, dx].T
        for j in range(2):
            base = 9600 + (dy * 2 + j) * 384
            lw3[0:64, base:base + 384] = w3[:, 128:192, dy, 2 * j].T
            lw3[64:128, base:base + 384] = w3[:, 128:192, dy, 2 * j + 1].T
        base = 13440 + dy * 384
        lw3[0:64, base:base + 384] = w3[:, 128:192, dy, 4].T
    lb3 = np.zeros((128, 3), f)
    lb3[:, 0] = b3[0:128]; lb3[:, 1] = b3[128:256]; lb3[:, 2] = b3[256:384]
    # conv4 / conv5: [128, (o, m)] with o = (dy*3+dx)*nkc + kc
    lw4 = np.zeros((128, 27 * 256), f)
    for dy in range(3):
        for dx in range(3):
            for kc in range(3):
                o = (dy * 3 + dx) * 3 + kc
                lw4[:, o * 256:(o + 1) * 256] = w4[:, kc * 128:(kc + 1) * 128, dy, dx].T
    lb4 = np.stack([b4[0:128], b4[128:256]], axis=1).astype(f)
    lw5 = np.zeros((128, 18 * 256), f)
    for dy in range(3):
        for dx in range(3):
            for kc in range(2):
                o = (dy * 3 + dx) * 2 + kc
                lw5[:, o * 256:(o + 1) * 256] = w5[:, kc * 128:(kc + 1) * 128, dy, dx].T
    lb5 = np.stack([b5[0:128], b5[128:256]], axis=1).astype(f)
    return dict(lw1=lw1.astype(BF), lw2=lw2.astype(BF), lb2=lb2,
                lw3=lw3.astype(BF), lb3=lb3, lw4=lw4.astype(BF), lb4=lb4,
                lw5=lw5.astype(BF), lb5=lb5)


def _prep_x13(x):
    """x [B,3,32,32] -> per-core [4, 100, 8*41*32] im2col-packed."""
    f = np.float32
    xpad = np.zeros((B, 3, 44, 42), f)
    xpad[:, :, 5:37, 5:37] = x
    X = np.zeros((100, B, 41, 32), f)
    for dyo in range(3):
        for dx in range(11):
            for c in range(3):
                X[dyo * 33 + dx * 3 + c] = xpad[:, c, dyo:dyo + 41, dx:dx + 32]
    X[99] = 1.0
    out = []
    for r in range(N_CORES):
        pc = X[:, r * BC:(r + 1) * BC]  # [100, 32, 41, 32]
        pc = pc.reshape(100, 4, 8 * 41 * 32).transpose(1, 0, 2)
        out.append(np.ascontiguousarray(pc).astype(BF))
    return out


def _prep_fc(fw1, fb1, fw2, fb2, fw3, fb3):
    f = np.float32
    outs = []
    for r in range(N_CORES):
        rq = r % 4  # position within the quad; quad = r // 4
        sl = slice(1024 * rq, 1024 * (rq + 1))
        # fw1s [128, (yx, cc, m)]: fw1[1024rq+m, (cc*128+k)*16+yx]
        fw1s = fw1[sl].reshape(1024, 2, 128, 16).transpose(2, 3, 1, 0).reshape(128, -1)
        fb1s = fb1[sl].reshape(8, 128).T
        # fw2s [128, (kc, m)]: fw2[1024rq+m, kc*128+k]
        fw2s = fw2[sl].reshape(1024, 32, 128).transpose(2, 1, 0).reshape(128, -1)
        fb2s = fb2[sl].reshape(8, 128).T
        # fw3s [128, (kc, m)]: fw3[m, 1024rq + kc*128 + k]
        fw3s = fw3[:, sl].reshape(100, 8, 128).transpose(2, 1, 0).reshape(128, -1)
        fb3s = (fb3 / 4).reshape(100, 1)
        outs.append(dict(
            fw1s=np.ascontiguousarray(fw1s).astype(BF),
            fb1s=np.ascontiguousarray(fb1s.astype(f)),
            fw2s=np.ascontiguousarray(fw2s).astype(BF),
            fb2s=np.ascontiguousarray(fb2s.astype(f)),
            fw3s=np.ascontiguousarray(fw3s).astype(BF),
            fb3s=np.ascontiguousarray(fb3s.astype(f)),
        ))
    return outs


_CACHE = {}

_SHAPES = dict(
    x13=(4, 100, 8 * 41 * 32), lw1=(100, 4 * 128),
    lw2=(128, 7 * 4 * 256), lb2=(128, 2),
    lw3=(128, 15360), lb3=(128, 3),
    lw4=(128, 27 * 256), lb4=(128, 2),
    lw5=(128, 18 * 256), lb5=(128, 2),
    fw1s=(128, 32 * 1024), fb1s=(128, 8),
    fw2s=(128, 32 * 1024), fb2s=(128, 8),
    fw3s=(128, 8 * 100), fb3s=(100, 1),
)


def _build():
    if "nc" in _CACHE:
        return _CACHE["nc"]
    nc = bacc.Bacc("TRN2", target_bir_lowering=False, debug=False,
                   num_devices=N_CORES)
    _BF16_INPUTS = {"x13", "lw1", "lw2", "lw3", "lw4", "lw5",
                    "fw1s", "fw2s", "fw3s"}
    t = {name: nc.dram_tensor(
            name, list(shape), BF16 if name in _BF16_INPUTS else F32,
            kind="ExternalInput").ap()
         for name, shape in _SHAPES.items()}
    yout = nc.dram_tensor("yout", [100, B], F32, kind="ExternalOutput").ap()
    with tile.TileContext(nc) as tc:
        _emit(nc, tc, t, yout)
    nc.compile()
    _CACHE["nc"] = nc
    return nc


def _in_maps(inputs):
    inputs = {k: np.asarray(v, np.float32) for k, v in inputs.items()}
    shared = _prep_shared(*[inputs[k] for k in
                            ("w1", "b1", "w2", "b2", "w3", "b3", "w4", "b4",
                             "w5", "b5")])
    x13s = _prep_x13(inputs["x"])
    fcs = _prep_fc(*[inputs[k] for k in
                     ("fw1", "fb1", "fw2", "fb2", "fw3", "fb3")])
    return [{**shared, "x13": x13s[r], **fcs[r]} for r in range(N_CORES)]


def kernel(x, w1, b1, w2, b2, w3, b3, w4, b4, w5, b5,
           fw1, fb1, fw2, fb2, fw3, fb3):
    nc = _build()
    in_maps = _in_maps(dict(x=x, w1=w1, b1=b1, w2=w2, b2=b2, w3=w3, b3=b3,
                            w4=w4, b4=b4, w5=w5, b5=b5, fw1=fw1, fb1=fb1,
                            fw2=fw2, fb2=fb2, fw3=fw3, fb3=fb3))
    res = run_bass_kernel_spmd(nc, in_maps, list(range(N_CORES)))
    y = res.results[0]["yout"]  # [100, 256]
    return np.ascontiguousarray(y.T)



# revision 12
# speedup vs baseline: 1.1639x; 1.1639x over previous
"""AlexNet-style CNN forward pass on 8 Trainium2 NeuronCores.

Strategy:
  - Convs data-parallel: batch 256 -> 32 per core, channels on partitions,
    conv = sum of shifted matmuls over kernel offsets (weights replicated).
  - conv1 (cin=3) uses host-packed im2col rows (3 dy-offsets x 11 dx x 3 ch
    + ones row for fused bias -> K=100) so the PE array is well utilized.
  - conv2 uses an x-shifted duplicate of its input (K=128 = 2 dx-offsets
    x 64 ch) to fill the contraction dim.
  - FC layers model-parallel: each core owns 512 rows of fc1/fc2 and 512
    K-columns of fc3; activations are AllGathered between layers, fc3
    partials AllReduced.  This cuts per-core FC weight DMA 8x.
  - Matmuls/activations run in bf16 (halves DMA + PE power so the clock
    stays unthrottled); PSUM accumulation + biases + fc3 AllReduce in fp32.
"""

import numpy as np
import ml_dtypes

BF = ml_dtypes.bfloat16

import concourse.bass as bass
import concourse.mybir as mybir
import concourse.tile as tile
from concourse import bacc
from concourse.bass_utils import run_bass_kernel_spmd

N_CORES = 8
B = 256
BC = B // N_CORES  # 32 images per core

F32 = mybir.dt.float32
BF16 = mybir.dt.bfloat16
RELU = mybir.ActivationFunctionType.Relu
IDENT = mybir.ActivationFunctionType.Identity


def _emit(nc, tc, t, yout):
    """Emit the whole network. t: dict name -> DRAM AP."""
    sync = nc.sync
    act = nc.scalar
    dve = nc.vector
    pool_e = nc.gpsimd

    psum = tc.alloc_tile_pool(name="psum", bufs=6, space="PSUM")
    scr = tc.alloc_tile_pool(name="scr", bufs=1, side="left")
    dram = tc.alloc_tile_pool(name="dram", bufs=1, space="DRAM")

    # ---------------- phase pools (queue alloc mode handles overlap) ----
    p_w12 = tc.alloc_tile_pool(name="p_w12", bufs=1, side="left")
    p_x2s = tc.alloc_tile_pool(name="p_x2s", bufs=1, side="left")
    p_x13 = tc.alloc_tile_pool(name="p_x13", bufs=3, side="left")

    # conv1+conv2 weights (host arrays already in SBUF layout).
    # Weights ride the Activation HWDGE queue so they never
    # head-of-line-block the x13 input stream on the SP queue.
    # conv1/conv2-mc1 weights have out-channels duplicated to M=128: the
    # "wasted" PE columns produce a second copy of the output, which the
    # maxpool writes at an x-offset of -1 -- giving conv2/conv3 their
    # x+1-shifted K-packing copies without any partition-shift DMA.
    lw1 = p_w12.tile([100, 4 * 128], BF16)
    act.dma_start(lw1[:], t["lw1"][:])
    lw2 = p_w12.tile([128, 7 * 4 * 256], BF16)
    act.dma_start(lw2[:], t["lw2"][:])
    lb2 = p_w12.tile([128, 2], F32)
    act.dma_start(lb2[:], t["lb2"][:])

    # warmup collectives: the first op of each replica-group shape pays a
    # ~40-100us cold-start; absorb them here, overlapped with conv1
    QUADS = [[0, 1, 2, 3], [4, 5, 6, 7]]
    PAIRS = [[0, 4], [1, 5], [2, 6], [3, 7]]
    wg_i = dram.tile([128, 8], BF16)
    wg_o = dram.tile([4, 128, 8], BF16)
    pool_e.collective_compute(
        "AllGather", mybir.AluOpType.bypass,
        replica_groups=QUADS, ins=[wg_i.opt()], outs=[wg_o.opt()])
    wr_i = dram.tile([128, 8], F32)
    wr_o = dram.tile([128, 8], F32)
    pool_e.collective_compute(
        "AllReduce", mybir.AluOpType.add,
        replica_groups=[list(range(N_CORES))],
        ins=[wr_i.opt()], outs=[wr_o.opt()])

    # conv2 input: [128, BC, 22, 23]; rows 0:64 ch c at x, rows 64:128 ch c at x+1
    X2s = p_x2s.tile([128, BC * 22 * 23], BF16)
    pool_e.memset(X2s[:], 0.0)

    def x2v(p0, p1, b0, nb, y0, ny, x0, nx):
        return X2s[p0:p1].rearrange("p (b y x) -> p b y x", b=BC, y=22, x=23)[
            :, b0:b0 + nb, y0:y0 + ny, x0:x0 + nx]

    # ---------------- conv1 + pool1 ----------------
    _sid = nc.enter_named_scope("L1_conv1", False)[0]
    for bg in range(4):  # groups of 8 images
        xt = p_x13.tile([100, 8 * 41 * 32], BF16, tag="x13")
        if bg == 0:  # split: first 2 images arrive ~4us sooner
            sync.dma_start(xt[:, :2 * 41 * 32], t["x13"][bg, :, :2 * 41 * 32])
            sync.dma_start(xt[:, 2 * 41 * 32:], t["x13"][bg, :, 2 * 41 * 32:])
        else:
            sync.dma_start(xt[:], t["x13"][bg])
        xtv = xt.rearrange("k (b y x) -> k b y x", b=8, y=41, x=32)
        for bl in range(8):
            b = bg * 8 + bl
            for h in range(2):  # vertical half of the 32x32 output
                ps = psum.tile([128, 512], F32, tag="ps")
                for pi, p in enumerate((0, 3, 6, 9)):
                    nc.tensor.matmul(
                        ps[:],
                        lw1[:, pi * 128:(pi + 1) * 128],
                        xtv[:, bl, h * 16 + p:h * 16 + p + 16, :],
                        start=(pi == 0), stop=(pi == 3),
                    )
                # evict+relu (bias came in via the ones-row), then 2x2 maxpool
                s1 = scr.tile([128, 512], BF16, tag="ev", bufs=3)
                act.activation(s1[:, :], ps[:], RELU)
                s1v = s1.rearrange("m (y x) -> m y x", y=16, x=32)
                m1 = scr.tile([128, 128], BF16, tag="m1", bufs=2)
                m2 = scr.tile([128, 128], BF16, tag="m2", bufs=2)
                dve.tensor_max(m1[:], s1v[:, 0::2, 0::2], s1v[:, 0::2, 1::2])
                dve.tensor_max(m2[:], s1v[:, 1::2, 0::2], s1v[:, 1::2, 1::2])
                y0 = h * 8 + 3
                m1v = m1.rearrange("m (y x) -> m y x", y=8, x=16)
                m2v = m2.rearrange("m (y x) -> m y x", y=8, x=16)
                dve.tensor_max(
                    x2v(0, 64, b, 1, y0, 8, 3, 16)[:, 0],
                    m1v[0:64], m2v[0:64])
                dve.tensor_max(
                    x2v(64, 128, b, 1, y0, 8, 2, 16)[:, 0],
                    m1v[64:128], m2v[64:128])
    p_x13.release()
    nc.leave_named_scope("L1_conv1", _sid, False)

    # conv3 weights (prefetch during conv2) + conv3 input buffers
    p_w3 = tc.alloc_tile_pool(name="p_w3", bufs=1, side="right")
    p_x3 = tc.alloc_tile_pool(name="p_x3", bufs=1, side="right")
    lw3 = p_w3.tile([128, 15360], BF16)
    act.dma_start(lw3[:], t["lw3"][:])
    lb3 = p_w3.tile([128, 3], F32)
    act.dma_start(lb3[:], t["lb3"][:])
    X3a = p_x3.tile([128, BC * 12 * 12], BF16)
    # X3b rows 64:128 hold the x+1-shifted copy of rows 0:64 (written
    # directly by conv2's duplicated-M maxpool), so kc1 runs as K=128
    # matmuls covering kernel-offset pairs (dy,2j)+(dy,2j+1)
    X3b = p_x3.tile([128, BC * 12 * 12], BF16)
    pool_e.memset(X3a[:], 0.0)
    pool_e.memset(X3b[:], 0.0)

    def x3v(xab, p0, p1, b0, nb, y0, ny, x0, nx):
        return xab[p0:p1].rearrange("p (b y x) -> p b y x", b=BC, y=12, x=12)[
            :, b0:b0 + nb, y0:y0 + ny, x0:x0 + nx]

    # ---------------- conv2 + pool2 ----------------
    _sid = nc.enter_named_scope("L2_conv2", False)[0]
    lw2v = lw2.rearrange("k (a j m) -> k a j m", a=7, j=4, m=256)
    for nt in range(16):  # pairs of images
        for mc in range(2):
            ps = psum.tile([128, 512], F32, tag="ps")
            first = True
            for dy in range(7):
                for j in range(4):
                    K = 128 if j < 3 else 64
                    xoff = 2 * j if j < 3 else 6
                    nc.tensor.matmul(
                        ps[:],
                        lw2v[0:K, dy, j, mc * 128:mc * 128 + 128],
                        x2v(0, K, nt * 2, 2, dy, 16, xoff, 16),
                        start=first, stop=(dy == 6 and j == 3),
                    )
                    first = False
            s2 = scr.tile([128, 512], BF16, tag="ev", bufs=3)
            act.activation(s2[:], ps[:], RELU, bias=lb2[:, mc:mc + 1])
            s2v = s2.rearrange("m (b y x) -> m b y x", b=2, y=16, x=16)
            m1 = scr.tile([128, 128], BF16, tag="m1", bufs=2)
            m2 = scr.tile([128, 128], BF16, tag="m2", bufs=2)
            dve.tensor_max(m1[:], s2v[:, :, 0::2, 0::2], s2v[:, :, 0::2, 1::2])
            dve.tensor_max(m2[:], s2v[:, :, 1::2, 0::2], s2v[:, :, 1::2, 1::2])
            m1v = m1.rearrange("m (b y x) -> m b y x", b=2, y=8, x=8)
            m2v = m2.rearrange("m (b y x) -> m b y x", b=2, y=8, x=8)
            if mc == 0:
                dve.tensor_max(x3v(X3a, 0, 128, nt * 2, 2, 2, 8, 2, 8), m1v[:], m2v[:])
            else:
                # ch 128:192 at (2,2) on rows 0:64; the duplicated copy goes
                # to rows 64:128 at x-offset 1 => x+1-shifted for kc1 pairing
                dve.tensor_max(x3v(X3b, 0, 64, nt * 2, 2, 2, 8, 2, 8),
                               m1v[0:64], m2v[0:64])
                dve.tensor_max(x3v(X3b, 64, 128, nt * 2, 2, 2, 8, 1, 8),
                               m1v[64:128], m2v[64:128])
    nc.leave_named_scope("L2_conv2", _sid, False)
    p_x2s.release()
    p_w12.release()

    # conv4/5 weights (prefetch during conv3) + conv4 input buffers
    p_w45 = tc.alloc_tile_pool(name="p_w45", bufs=1, side="left")
    p_x4 = tc.alloc_tile_pool(name="p_x4", bufs=1, side="left")
    lw4 = p_w45.tile([128, 27 * 256], BF16)
    act.dma_start(lw4[:], t["lw4"][:])
    lb4 = p_w45.tile([128, 2], F32)
    act.dma_start(lb4[:], t["lb4"][:])
    lw5 = p_w45.tile([128, 18 * 256], BF16)
    act.dma_start(lw5[:], t["lw5"][:])
    lb5 = p_w45.tile([128, 2], F32)
    act.dma_start(lb5[:], t["lb5"][:])
    X4 = []
    for i in range(3):
        X4.append(p_x4.tile([128, BC * 10 * 10], BF16, name=f"X4_{i}"))
        pool_e.memset(X4[i][:], 0.0)

    def xv10(xab, p0, p1, b0, nb, y0, ny, x0, nx):
        return xab[p0:p1].rearrange("p (b y x) -> p b y x", b=BC, y=10, x=10)[
            :, b0:b0 + nb, y0:y0 + ny, x0:x0 + nx]

    _sid = nc.enter_named_scope("L3_conv3", False)[0]
    # ---------------- conv3 ----------------
    for nt in range(4):  # 8 images
        for mc in range(3):
            ps = psum.tile([128, 512], F32, tag="ps")
            first = True
            for dy in range(5):
                for dx in range(5):
                    blk = dy * 5 + dx
                    nc.tensor.matmul(
                        ps[:],
                        lw3[0:128, blk * 384 + mc * 128:blk * 384 + mc * 128 + 128],
                        x3v(X3a, 0, 128, nt * 8, 8, dy, 8, dx, 8),
                        start=first, stop=False,
                    )
                    first = False
            for dy in range(5):
                for j in range(2):
                    base = 9600 + (dy * 2 + j) * 384 + mc * 128
                    nc.tensor.matmul(
                        ps[:],
                        lw3[0:128, base:base + 128],
                        x3v(X3b, 0, 128, nt * 8, 8, dy, 8, 2 * j, 8),
                        start=False, stop=False,
                    )
                base = 13440 + dy * 384 + mc * 128
                nc.tensor.matmul(
                    ps[:],
                    lw3[0:64, base:base + 128],
                    x3v(X3b, 0, 64, nt * 8, 8, dy, 8, 4, 8),
                    start=False, stop=(dy == 4),
                )
            act.activation(
                xv10(X4[mc], 0, 128, nt * 8, 8, 1, 8, 1, 8),
                ps.rearrange("m (b y x) -> m b y x", b=8, y=8, x=8),
                RELU, bias=lb3[:, mc:mc + 1])
    nc.leave_named_scope("L3_conv3", _sid, False)
    p_x3.release()
    p_w3.release()

    # fc1 weights (prefetch during conv4) + conv5 input buffers
    p_fw1 = tc.alloc_tile_pool(name="p_fw1", bufs=1, side="right")
    p_fw2 = tc.alloc_tile_pool(name="p_fw2", bufs=1, side="right")
    p_x5 = tc.alloc_tile_pool(name="p_x5", bufs=1, side="right")
    fw1 = p_fw1.tile([128, 32 * 1024], BF16)
    act.dma_start(fw1[:], t["fw1s"][:])
    fb1 = p_fw1.tile([128, 8], F32)
    act.dma_start(fb1[:], t["fb1s"][:])
    fw2 = p_fw2.tile([128, 32 * 1024], BF16)
    act.dma_start(fw2[:], t["fw2s"][:])
    fb2 = p_fw2.tile([128, 8], F32)
    act.dma_start(fb2[:], t["fb2s"][:])
    fw3 = p_fw2.tile([128, 8 * 100], BF16)
    act.dma_start(fw3[:], t["fw3s"][:])
    fb3 = p_fw2.tile([100, 1], F32)
    act.dma_start(fb3[:], t["fb3s"][:])
    qsel = p_fw2.tile([128, 2], F32)
    act.dma_start(qsel[:], t["qsel"][:])
    X5 = []
    for i in range(2):
        X5.append(p_x5.tile([128, BC * 10 * 10], BF16, name=f"X5_{i}"))
        pool_e.memset(X5[i][:], 0.0)

    # pool5 output, separate tiles per image-half so the half-0 gather
    # trigger doesn't wait on half-1 pool writes
    p_p5 = tc.alloc_tile_pool(name="p_p5", bufs=1, side="left")
    P5 = [[p_p5.tile([128, (BC // 2) * 16], BF16, name=f"P5_{h}_{i}")
           for i in range(2)] for h in range(2)]

    # DRAM staging for the pipelined gather/FC tail.  FC layers are
    # sharded over 4-core quads (cores 0-3 already hold images 0:128, so
    # each quad only gathers ITS half of the batch -- half the collective
    # bytes, and the two quads' collectives run concurrently).  Within a
    # quad each core owns 1024 fc1/fc2 rows and 1024 fc3 K-columns; a
    # single masked all-core AllReduce assembles the final logits.
    HB = BC // 2          # 16 local images per half
    NHQ = 4 * HB          # 64 quad images per half
    cin5 = [dram.tile([2, 128, HB * 16], BF16, name=f"cin5_{h}") for h in range(2)]
    g1 = [dram.tile([4, 2, 128, HB * 16], BF16, name=f"g1_{h}")
          for h in range(2)]

    def stage_gather1(h):
        for mc in range(2):
            sync.dma_start(cin5[h][mc], P5[h][mc][:])
        pool_e.collective_compute(
            "AllGather", mybir.AluOpType.bypass,
            replica_groups=QUADS, ins=[cin5[h].opt()], outs=[g1[h].opt()])

    _sid = nc.enter_named_scope("L45_conv45", False)[0]
    # ------- conv4 + conv5 + pool5, interleaved by image-half so the
    # ------- half-0 gather hides under half-1's compute -------
    lw4v = lw4.rearrange("k (o m) -> k o m", o=27)
    lw5v = lw5.rearrange("k (o m) -> k o m", o=18)
    for hh in range(2):
        for nt in (2 * hh, 2 * hh + 1):
            for mc in range(2):
                ps = psum.tile([128, 512], F32, tag="ps")
                first = True
                for dy in range(3):
                    for dx in range(3):
                        for kc in range(3):
                            o = (dy * 3 + dx) * 3 + kc
                            nc.tensor.matmul(
                                ps[:],
                                lw4v[:, o, mc * 128:mc * 128 + 128],
                                xv10(X4[kc], 0, 128, nt * 8, 8, dy, 8, dx, 8),
                                start=first, stop=(o == 26),
                            )
                            first = False
                act.activation(
                    xv10(X5[mc], 0, 128, nt * 8, 8, 1, 8, 1, 8),
                    ps.rearrange("m (b y x) -> m b y x", b=8, y=8, x=8),
                    RELU, bias=lb4[:, mc:mc + 1])
        for nt in (2 * hh, 2 * hh + 1):
            for mc in range(2):
                ps = psum.tile([128, 512], F32, tag="ps")
                first = True
                for dy in range(3):
                    for dx in range(3):
                        for kc in range(2):
                            o = (dy * 3 + dx) * 2 + kc
                            nc.tensor.matmul(
                                ps[:],
                                lw5v[:, o, mc * 128:mc * 128 + 128],
                                xv10(X5[kc], 0, 128, nt * 8, 8, dy, 8, dx, 8),
                                start=first, stop=(o == 17),
                            )
                            first = False
                s5 = scr.tile([128, 512], BF16, tag="ev", bufs=3)
                act.activation(s5[:], ps[:], RELU, bias=lb5[:, mc:mc + 1])
                s5v = s5.rearrange("m (b y x) -> m b y x", b=8, y=8, x=8)
                m1 = scr.tile([128, 128], BF16, tag="m1", bufs=2)
                m2 = scr.tile([128, 128], BF16, tag="m2", bufs=2)
                dve.tensor_max(m1[:], s5v[:, :, 0::2, 0::2], s5v[:, :, 0::2, 1::2])
                dve.tensor_max(m2[:], s5v[:, :, 1::2, 0::2], s5v[:, :, 1::2, 1::2])
                p5v = P5[hh][mc].rearrange("p (b y x) -> p b y x", b=HB, y=4, x=4)
                bo = (nt % 2) * 8
                dve.tensor_max(
                    p5v[:, bo:bo + 8, :, :],
                    m1.rearrange("m (b y x) -> m b y x", b=8, y=4, x=4),
                    m2.rearrange("m (b y x) -> m b y x", b=8, y=4, x=4))
        stage_gather1(hh)
    nc.leave_named_scope("L45_conv45", _sid, False)
    p_p5.release()
    p_x4.release()
    p_w45.release()
    p_x5.release()

    _sid = nc.enter_named_scope("G1_gather", False)[0]
    # un-interleave gathered pool5 into SBUF, then re-layout so each fc1
    # K-step reads a CONTIGUOUS rhs block (strided cols cross an SBUF
    # cacheline per column and throttle PE streaming ~4x)
    p_h1 = tc.alloc_tile_pool(name="p_h1", bufs=1, side="right")
    H1 = [[p_h1.tile([128, 4 * HB * 16], BF16, name=f"H1_{h}_{cc}")
           for cc in range(2)] for h in range(2)]
    H1y = [p_h1.tile([128, 32 * NHQ], BF16, name=f"H1y_{h}") for h in range(2)]
    for h in range(2):
        for cc in range(2):
            sync.dma_start(
                H1[h][cc].rearrange("c (r f) -> c r f", r=4),
                g1[h][:, cc].rearrange("r c f -> c r f"))
        for cc in range(2):
            dve.tensor_copy(
                H1y[h].rearrange("c (y s q) -> c y s q", y=16, s=2)[:, :, cc],
                H1[h][cc].rearrange("c (q y) -> c y q", y=16))
    nc.leave_named_scope("G1_gather", _sid, False)

    _sid = nc.enter_named_scope("F1_fc1", False)[0]
    # ------------- fc1 (quad-model-parallel over 1024 outputs) -------------
    p_f1 = tc.alloc_tile_pool(name="p_f1", bufs=1, side="left")
    F1 = [p_f1.tile([128, 8 * NHQ], BF16, name=f"F1_{h}") for h in range(2)]
    cin6 = [dram.tile([128, 8 * NHQ], BF16, name=f"cin6_{h}") for h in range(2)]
    g2 = [dram.tile([4, 128, 8 * NHQ], BF16, name=f"g2_{h}")
          for h in range(2)]
    fw1v = fw1.rearrange("k (y c m) -> k y c m", y=16, c=2, m=1024)
    for h in range(2):
        rhsv = H1y[h].rearrange("c (y s q) -> c y s q", y=16, s=2)
        for mc in range(8):
            ps = psum.tile([128, NHQ], F32, tag="ps")
            first = True
            for yx in range(16):
                for cc in range(2):
                    nc.tensor.matmul(
                        ps[:],
                        fw1v[:, yx, cc, mc * 128:mc * 128 + 128],
                        rhsv[:, yx, cc],
                        start=first, stop=(yx == 15 and cc == 1))
                    first = False
            act.activation(F1[h][:, mc * NHQ:(mc + 1) * NHQ], ps[:], RELU,
                           bias=fb1[:, mc:mc + 1])
        sync.dma_start(cin6[h][:], F1[h][:])
        pool_e.collective_compute(
            "AllGather", mybir.AluOpType.bypass,
            replica_groups=QUADS, ins=[cin6[h].opt()], outs=[g2[h].opt()])
    p_h1.release()
    nc.leave_named_scope("F1_fc1", _sid, False)

    _sid = nc.enter_named_scope("G2_gather", False)[0]
    # un-interleave gathered fc1: feature block kc = rq*8 + mc, so the
    # (rq, mc, b) column order is already kc-major
    p_h2 = tc.alloc_tile_pool(name="p_h2", bufs=1, side="right")
    H2 = [p_h2.tile([128, 32 * NHQ], BF16, name=f"H2_{h}") for h in range(2)]
    for h in range(2):
        sync.dma_start(
            H2[h].rearrange("c (r f) -> c r f", r=4),
            g2[h].rearrange("r c f -> c r f"))
    p_f1.release()
    nc.leave_named_scope("G2_gather", _sid, False)

    _sid = nc.enter_named_scope("F2_fc23", False)[0]
    # ---------------- fc2 + fc3 partial + masked AllReduce ----------------
    # Each core writes its fc3 partial into BOTH quad slots of s4, scaled
    # by qsel (1 for its own quad, 0 for the other); one all-core
    # AllReduce then assembles the full [100, 256] logits everywhere.
    p_f2 = tc.alloc_tile_pool(name="p_f2", bufs=1, side="left")
    F2 = [p_f2.tile([128, 8 * NHQ], BF16, name=f"F2_{h}") for h in range(2)]
    s4 = p_f2.tile([100, B], F32, name="s4")
    cin7 = dram.tile([100, B], F32)
    g3 = dram.tile([100, B], F32)
    youtv = yout.rearrange("m (q rr g) -> m q rr g", q=2, rr=4)
    fw2v = fw2.rearrange("k (a m) -> k a m", a=32)
    fw3v = fw3.rearrange("k (a m) -> k a m", a=8)
    for h in range(2):
        for mc in range(8):
            ps = psum.tile([128, NHQ], F32, tag="ps")
            for kc in range(32):
                nc.tensor.matmul(
                    ps[:], fw2v[:, kc, mc * 128:mc * 128 + 128],
                    H2[h][:, kc * NHQ:(kc + 1) * NHQ],
                    start=(kc == 0), stop=(kc == 31))
            act.activation(F2[h][:, mc * NHQ:(mc + 1) * NHQ], ps[:], RELU,
                           bias=fb2[:, mc:mc + 1])
        ps3 = psum.tile([128, NHQ], F32, tag="ps")
        for kc in range(8):
            nc.tensor.matmul(
                ps3[0:100, :], fw3v[:, kc, :], F2[h][:, kc * NHQ:(kc + 1) * NHQ],
                start=(kc == 0), stop=(kc == 7))
        for q in range(2):
            act.activation(s4[:, q * 128 + h * NHQ:q * 128 + (h + 1) * NHQ],
                           ps3[0:100, :], IDENT, bias=fb3[:],
                           scale=qsel[0:100, q:q + 1])
        sync.dma_start(cin7[:, h * NHQ:(h + 1) * NHQ], s4[:, h * NHQ:(h + 1) * NHQ])
        sync.dma_start(cin7[:, 128 + h * NHQ:128 + (h + 1) * NHQ],
                       s4[:, 128 + h * NHQ:128 + (h + 1) * NHQ])
    pool_e.collective_compute(
        "AllReduce", mybir.AluOpType.add,
        replica_groups=[list(range(N_CORES))],
        ins=[cin7.opt()], outs=[g3.opt()])
    # yout col = q*128 + rq*32 + h*16 + b; g3 col = q*128 + h*64 + rq*16 + b
    for h in range(2):
        for q in range(2):
            sync.dma_start(
                youtv[:, q, :, h * HB:(h + 1) * HB],
                g3[:, q * 128 + h * NHQ:q * 128 + (h + 1) * NHQ]
                .rearrange("m (rr b) -> m rr b", rr=4))
    nc.leave_named_scope("F2_fc23", _sid, False)
    p_f2.release()
    p_h2.release()
    p_fw2.release()
    p_fw1.release()

    scr.release()
    dram.release()
    psum.release()


# ---------------------------------------------------------------------------
# host-side input prep (numpy; all weight arrays already in SBUF layout)
# ---------------------------------------------------------------------------

def _prep_shared(w1, b1, w2, b2, w3, b3, w4, b4, w5, b5):
    f = np.float32
    # conv1: rows r = dyo*33 + dx*3 + c, row 99 = bias(ones); 4 passes
    # dy=3p+dyo; out-channels duplicated to M=128 (cols 64:128 = cols 0:64)
    lw1 = np.zeros((100, 4 * 128), f)
    for p in range(4):
        for dyo in range(3):
            dy = 3 * p + dyo
            if dy > 10:
                continue
            for dx in range(11):
                for c in range(3):
                    lw1[dyo * 33 + dx * 3 + c, p * 128:p * 128 + 64] = w1[:, c, dy, dx]
    lw1[99, 0:64] = b1
    lw1[:, :] = lw1.reshape(100, 4, 2, 64)[:, :, 0:1].repeat(2, 2).reshape(100, -1)
    # conv2: [128, (dy,j,m)] m=256: j<3 -> rows s*64+c = ch c at dx=2j+s;
    # j=3 -> dx=6.  m 0:128 = out-ch 0:128; m 128:256 = out-ch 128:192 twice
    lw2 = np.zeros((128, 7 * 4 * 256), f)
    w2m = np.concatenate([w2[0:128], w2[128:192], w2[128:192]], axis=0)  # [256,...]
    for dy in range(7):
        for j in range(3):
            for s in range(2):
                lw2[s * 64:(s + 1) * 64, (dy * 4 + j) * 256:(dy * 4 + j + 1) * 256] = \
                    w2m[:, :, dy, 2 * j + s].T
        lw2[0:64, (dy * 4 + 3) * 256:(dy * 4 + 4) * 256] = w2m[:, :, dy, 6].T
    lb2 = np.zeros((128, 2), f)
    lb2[:, 0] = b2[0:128]
    lb2[0:64, 1] = b2[128:192]
    lb2[64:128, 1] = b2[128:192]
    # conv3: [128, 15360]: kc0 cols blk*384+m; kc1 pairs (dy,2j)+(dy,2j+1)
    # at 9600+(dy*2+j)*384 rows [0:64|64:128]; kc1 singles (dy,4) at
    # 13440+dy*384 rows 0:64
    lw3 = np.zeros((128, 15360), f)
    for dy in range(5):
        for dx in range(5):
            blk = dy * 5 + dx
            lw3[:, blk * 384:(blk + 1) * 384] = w3[:, 0:128, dy, dx].T
        for j in range(2):
            base = 9600 + (dy * 2 + j) * 384
            lw3[0:64, base:base + 384] = w3[:, 128:192, dy, 2 * j].T
            lw3[64:128, base:base + 384] = w3[:, 128:192, dy, 2 * j + 1].T
        base = 13440 + dy * 384
        lw3[0:64, base:base + 384] = w3[:, 128:192, dy, 4].T
    lb3 = np.zeros((128, 3), f)
    lb3[:, 0] = b3[0:128]; lb3[:, 1] = b3[128:256]; lb3[:, 2] = b3[256:384]
    # conv4 / conv5: [128, (o, m)] with o = (dy*3+dx)*nkc + kc
    lw4 = np.zeros((128, 27 * 256), f)
    for dy in range(3):
        for dx in range(3):
            for kc in range(3):
                o = (dy * 3 + dx) * 3 + kc
                lw4[:, o * 256:(o + 1) * 256] = w4[:, kc * 128:(kc + 1) * 128, dy, dx].T
    lb4 = np.stack([b4[0:128], b4[128:256]], axis=1).astype(f)
    lw5 = np.zeros((128, 18 * 256), f)
    for dy in range(3):
        for dx in range(3):
            for kc in range(2):
                o = (dy * 3 + dx) * 2 + kc
                lw5[:, o * 256:(o + 1) * 256] = w5[:, kc * 128:(kc + 1) * 128, dy, dx].T
    lb5 = np.stack([b5[0:128], b5[128:256]], axis=1).astype(f)
    return dict(lw1=lw1.astype(BF), lw2=lw2.astype(BF), lb2=lb2,
                lw3=lw3.astype(BF), lb3=lb3, lw4=lw4.astype(BF), lb4=lb4,
                lw5=lw5.astype(BF), lb5=lb5)


def _prep_x13(x):
    """x [B,3,32,32] -> per-core [4, 100, 8*41*32] im2col-packed."""
    f = np.float32
    xpad = np.zeros((B, 3, 44, 42), f)
    xpad[:, :, 5:37, 5:37] = x
    X = np.zeros((100, B, 41, 32), f)
    for dyo in range(3):
        for dx in range(11):
            for c in range(3):
                X[dyo * 33 + dx * 3 + c] = xpad[:, c, dyo:dyo + 41, dx:dx + 32]
    X[99] = 1.0
    out = []
    for r in range(N_CORES):
        pc = X[:, r * BC:(r + 1) * BC]  # [100, 32, 41, 32]
        pc = pc.reshape(100, 4, 8 * 41 * 32).transpose(1, 0, 2)
        out.append(np.ascontiguousarray(pc).astype(BF))
    return out


def _prep_fc(fw1, fb1, fw2, fb2, fw3, fb3):
    f = np.float32
    outs = []
    for r in range(N_CORES):
        rq = r % 4  # position within the quad; quad = r // 4
        sl = slice(1024 * rq, 1024 * (rq + 1))
        # fw1s [128, (yx, cc, m)]: fw1[1024rq+m, (cc*128+k)*16+yx]
        fw1s = fw1[sl].reshape(1024, 2, 128, 16).transpose(2, 3, 1, 0).reshape(128, -1)
        fb1s = fb1[sl].reshape(8, 128).T
        # fw2s [128, (kc, m)]: fw2[1024rq+m, kc*128+k]
        fw2s = fw2[sl].reshape(1024, 32, 128).transpose(2, 1, 0).reshape(128, -1)
        fb2s = fb2[sl].reshape(8, 128).T
        # fw3s [128, (kc, m)]: fw3[m, 1024rq + kc*128 + k]
        fw3s = fw3[:, sl].reshape(100, 8, 128).transpose(2, 1, 0).reshape(128, -1)
        fb3s = (fb3 / 4).reshape(100, 1)
        qsel = np.zeros((128, 2), f)
        qsel[:, 0 if r < 4 else 1] = 1.0
        outs.append(dict(
            fw1s=np.ascontiguousarray(fw1s).astype(BF),
            fb1s=np.ascontiguousarray(fb1s.astype(f)),
            fw2s=np.ascontiguousarray(fw2s).astype(BF),
            fb2s=np.ascontiguousarray(fb2s.astype(f)),
            fw3s=np.ascontiguousarray(fw3s).astype(BF),
            fb3s=np.ascontiguousarray(fb3s.astype(f)),
            qsel=np.ascontiguousarray(qsel),
        ))
    return outs


_CACHE = {}

_SHAPES = dict(
    x13=(4, 100, 8 * 41 * 32), lw1=(100, 4 * 128),
    lw2=(128, 7 * 4 * 256), lb2=(128, 2),
    lw3=(128, 15360), lb3=(128, 3),
    lw4=(128, 27 * 256), lb4=(128, 2),
    lw5=(128, 18 * 256), lb5=(128, 2),
    fw1s=(128, 32 * 1024), fb1s=(128, 8),
    fw2s=(128, 32 * 1024), fb2s=(128, 8),
    fw3s=(128, 8 * 100), fb3s=(100, 1), qsel=(128, 2),
)


def _build():
    if "nc" in _CACHE:
        return _CACHE["nc"]
    nc = bacc.Bacc("TRN2", target_bir_lowering=False, debug=False,
                   num_devices=N_CORES)
    _BF16_INPUTS = {"x13", "lw1", "lw2", "lw3", "lw4", "lw5",
                    "fw1s", "fw2s", "fw3s"}
    t = {name: nc.dram_tensor(
            name, list(shape), BF16 if name in _BF16_INPUTS else F32,
            kind="ExternalInput").ap()
         for name, shape in _SHAPES.items()}
    yout = nc.dram_tensor("yout", [100, B], F32, kind="ExternalOutput").ap()
    with tile.TileContext(nc) as tc:
        _emit(nc, tc, t, yout)
    nc.compile()
    _CACHE["nc"] = nc
    return nc


def _in_maps(inputs):
    inputs = {k: np.asarray(v, np.float32) for k, v in inputs.items()}
    shared = _prep_shared(*[inputs[k] for k in
                            ("w1", "b1", "w2", "b2", "w3", "b3", "w4", "b4",
                             "w5", "b5")])
    x13s = _prep_x13(inputs["x"])
    fcs = _prep_fc(*[inputs[k] for k in
                     ("fw1", "fb1", "fw2", "fb2", "fw3", "fb3")])
    return [{**shared, "x13": x13s[r], **fcs[r]} for r in range(N_CORES)]


def kernel(x, w1, b1, w2, b2, w3, b3, w4, b4, w5, b5,
           fw1, fb1, fw2, fb2, fw3, fb3):
    nc = _build()
    in_maps = _in_maps(dict(x=x, w1=w1, b1=b1, w2=w2, b2=b2, w3=w3, b3=b3,
                            w4=w4, b4=b4, w5=w5, b5=b5, fw1=fw1, fb1=fb1,
                            fw2=fw2, fb2=fb2, fw3=fw3, fb3=fb3))
    res = run_bass_kernel_spmd(nc, in_maps, list(range(N_CORES)))
    y = res.results[0]["yout"]  # [100, 256]
    return np.ascontiguousarray(y.T)

